# revision 35
# baseline (speedup 1.0000x reference)
"""Trainium2 Bass kernel for the CAM (cross-attention module) problem.

Math (per sample b):
    img = w_img @ x1_b          # [256, 4096]
    kv  = w_txt @ x2_b          # [256, 4096]
    attn = softmax(img @ kv^T)  # [256, 256], softmax over last dim
    y = gamma * (attn @ kv) + img
    out_b = w_out @ y           # [512, 4096]

Sharding: data-parallel over batch, 16 samples -> 2 per core x 8 cores,
no collectives.

Precision: all matmuls run in float32r (TRN2's single-pass fp32 matmul
mode, ~1.5e-4 per-product vs 1e-7 for the 2x-slower LOW_HIGH fp32 mode).
End-to-end relative error vs the fp32 reference is ~1.3e-3.

Layout strategy: the spatial contraction (attn logits) needs
spatial-major operands while the residual + output conv need
channel-major ones.  img/kv are computed channel-major from the natural
HBM layout, and the spatial-major copies are made with PE transposes
(one 128x128 transpose per block - far cheaper than re-contracting over
input channels).  The two samples per core are software-pipelined: each
sample's tail (softmax + attn@kv + output conv) is interleaved into the
next sample's projection chunks so the tensor engine never drains.
"""

import numpy as np

# Problem shapes (hardcoded per the harness contract)
B = 16
C1 = 512          # x1 channels (also output channels)
C2 = 320          # x2 channels
C2P = 384         # x2 channels padded to a multiple of 128 (K<128 matmuls
                  # and partition-offset memsets are both broken on HW)
C = 256           # projected channels
HW = 64 * 64      # spatial size
NCORES = 8
SPC = B // NCORES  # samples per core

_BUILD_CACHE = {}


def _nonce_len():
    import inspect
    import zlib
    return 2 + (zlib.crc32(inspect.getsource(_build_nc).encode()) % 997)


def _build_nc(spc=SPC, c1=C1, c2p=C2P, c=C, hw=HW, ch=512):
    """Build the per-core Bass program (same program on all cores)."""
    import concourse.tile as tile
    from concourse import bacc, mybir

    f32 = mybir.dt.float32
    f32r = mybir.dt.float32r
    P = 128
    K1 = c1 // P           # k-tiles of x1 channels
    K2 = c2p // P          # k-tiles of x2 channels (padded)
    KC = c // P            # k-tiles of projected channels
    MO = c1 // P           # m-tiles of output conv
    NCH = hw // ch         # spatial chunks
    TPC = ch // P          # 128-wide spatial tiles per chunk

    # Bacc (not plain Bass): its compile() runs move_matmul_waits_to_ldweights
    # + generate_event_semaphores, without which walrus rejects any Matmult
    # carrying more than one semaphore wait.
    nc = bacc.Bacc("TRN2", target_bir_lowering=False)
    x1 = nc.declare_dram_parameter("x1", [spc, c1, hw], f32r, isOutput=False)
    x2 = nc.declare_dram_parameter("x2", [spc, c2p, hw], f32r, isOutput=False)
    wiT = nc.declare_dram_parameter("w_imgT", [c1, c], f32r, isOutput=False)
    wtT = nc.declare_dram_parameter("w_txtT", [c2p, c], f32r, isOutput=False)
    woT = nc.declare_dram_parameter("w_outT", [c, c1], f32r, isOutput=False)
    gamma = nc.declare_dram_parameter("gamma", [1], f32, isOutput=False)
    idin = nc.declare_dram_parameter("ident", [P, P], f32r, isOutput=False)
    # The PJRT executable cache fingerprints the HLO without the embedded
    # BIR payload, so two different kernels with identical I/O signatures
    # collide. A source-hash-sized dummy input makes the signature unique.
    nc.declare_dram_parameter("nonce", [1, _nonce_len()], f32, isOutput=False)
    out = nc.declare_dram_parameter("out", [spc, c1, hw], f32, isOutput=True)

    Exp = mybir.ActivationFunctionType.Exp
    X = mybir.AxisListType.X

    with (
        tile.TileContext(nc) as tc,
        tc.tile_pool(name="singles", bufs=1) as singles,
        tc.tile_pool(name="xin", bufs=2) as xin,
        tc.tile_pool(name="tch", bufs=2) as tch,
        tc.tile_pool(name="imgp", bufs=2) as imgp,
        tc.tile_pool(name="kvp", bufs=NCH + 2) as kvp,
        tc.tile_pool(name="attnsb", bufs=2) as attnsb,
        tc.tile_pool(name="smalls", bufs=4) as smalls,
        tc.tile_pool(name="ostage", bufs=4) as ostage,
        tc.tile_pool(name="psA", bufs=4, space="PSUM") as psA,
        tc.tile_pool(name="psB", bufs=2, space="PSUM") as psB,
        tc.tile_pool(name="psAttn", bufs=2, space="PSUM") as psAttn,
    ):
        # ---- constants. Weight loads for chunk 0 are emitted interleaved
        # with the chunk-0 x loads (see passA_chunk) so the SP queue issues
        # the first matmul's inputs as early as possible; woT/ident/gamma
        # are deferred past the first chunk since nothing needs them early.
        wiT_sb = singles.tile([P, K1, c], f32r)
        wtT_sb = singles.tile([P, K2, c], f32r)
        woT_sb = singles.tile([P, KC, c1], f32r)
        ident = singles.tile([P, P], f32r)
        gamma_sb = singles.tile([P, 1], f32)

        def emit_deferred_constants():
            for k in range(KC):
                nc.sync.dma_start(out=woT_sb[:, k, :], in_=woT[k * P:(k + 1) * P, :])
            nc.sync.dma_start(out=ident, in_=idin[:])
            nc.sync.dma_start(out=gamma_sb, in_=gamma[:].to_broadcast((P, 1)))

        # ---- per-sample emission helpers -------------------------------
        def passA_chunk(st, s, cc, first=False):
            # `first`: interleave the weight loads with the chunk-0 x loads
            # so the SP queue issues the first matmul's inputs earliest.
            cs = cc * ch
            x1c = xin.tile([P, K1, ch], f32r, tag="x1c", name="x1c", bufs=3)
            for k in range(K1):
                nc.sync.dma_start(out=x1c[:, k, :],
                                  in_=x1[s, k * P:(k + 1) * P, cs:cs + ch])
                if first:
                    nc.sync.dma_start(out=wiT_sb[:, k, :],
                                      in_=wiT[k * P:(k + 1) * P, :])
            x2c = xin.tile([P, K2, ch], f32r, tag="x2c", name="x2c", bufs=3)
            for k in range(K2):
                kp = min(c2p - k * P, P)
                nc.sync.dma_start(out=x2c[:kp, k, :],
                                  in_=x2[s, k * P:k * P + kp, cs:cs + ch])
                if first:
                    nc.sync.dma_start(out=wtT_sb[:kp, k, :],
                                      in_=wtT[k * P:k * P + kp, :])
            for m in range(KC):
                ps = psA.tile([P, ch], f32, tag="a", name="ps_img")
                for k in range(K1):
                    nc.tensor.matmul(ps, lhsT=wiT_sb[:, k, m * P:(m + 1) * P],
                                     rhs=x1c[:, k, :],
                                     start=(k == 0), stop=(k == K1 - 1))
                nc.vector.tensor_copy(out=st["img"][:, m, cs:cs + ch], in_=ps)
            kvt = kvp.tile([P, KC, ch], f32r, tag="kv", name="kvt")
            st["kvch"][cc] = kvt
            for m in range(KC):
                ps = psA.tile([P, ch], f32, tag="a", name="ps_kv")
                for k in range(K2):
                    nc.tensor.matmul(ps, lhsT=wtT_sb[:, k, m * P:(m + 1) * P],
                                     rhs=x2c[:, k, :],
                                     start=(k == 0), stop=(k == K2 - 1))
                nc.vector.tensor_copy(out=kvt[:, m, :], in_=ps)

        def transposes(st, s, pc):
            # spatial-major orientations via PE transpose of img / kv chunks
            pcs = pc * ch
            imgT_c = tch.tile([P, TPC, c], f32r, tag="imgT", name="imgT_c")
            for t in range(TPC):
                ps = psB.tile([P, c], f32r, tag="b", name="ps_imgT")
                for i in range(KC):
                    nc.tensor.transpose(
                        ps[:, i * P:(i + 1) * P],
                        st["img"][:, i, pcs + t * P:pcs + (t + 1) * P], ident)
                nc.scalar.copy(out=imgT_c[:, t, :], in_=ps)
            txtT_c = tch.tile([P, TPC, c], f32r, tag="txtT", name="txtT_c")
            for t in range(TPC):
                ps = psB.tile([P, c], f32r, tag="b", name="ps_txtT")
                for i in range(KC):
                    nc.tensor.transpose(
                        ps[:, i * P:(i + 1) * P],
                        st["kvch"][pc][:, i, t * P:(t + 1) * P], ident)
                nc.scalar.copy(out=txtT_c[:, t, :], in_=ps)
            st["imgT"][pc] = imgT_c
            st["txtT"][pc] = txtT_c

        def attn_chunk(st, s, pc):
            if st["attn_ps"] is None:
                st["attn_ps"] = [
                    psAttn.tile([P, c], f32, tag="attn", name=f"attn{s}_{m}")
                    for m in range(KC)
                ]
            for m in range(KC):
                for t in range(TPC):
                    nc.tensor.matmul(
                        st["attn_ps"][m],
                        lhsT=st["imgT"][pc][:, t, m * P:(m + 1) * P],
                        rhs=st["txtT"][pc][:, t, :],
                        start=(pc == 0 and t == 0),
                        stop=(pc == NCH - 1 and t == TPC - 1))
            st["imgT"][pc] = st["txtT"][pc] = None

        def softmax(st, s, cover=()):
            # softmax over the free (d) axis, gamma folded in; transpose to
            # attnT [d, c] for the attn@kv contraction.  `cover` closures are
            # emitted between the DVE/ACT stats and the PE transposes so the
            # tensor engine has work while the serial softmax chain runs.
            attnT_sb = attnsb.tile([P, KC, c], f32r, tag="attnT", name="attnT")
            st["attnT"] = attnT_sb
            exps = []
            for m in range(KC):
                nmax = smalls.tile([P, 1], f32, tag="nmax", name="nmax")
                nc.vector.reduce_max(out=nmax, in_=st["attn_ps"][m], axis=X,
                                     negate=True)
                exp_sb = smalls.tile([P, c], f32r, tag="exp", name="exp_sb")
                rsum = smalls.tile([P, 1], f32, tag="rsum", name="rsum")
                nc.scalar.activation(out=exp_sb, in_=st["attn_ps"][m], func=Exp,
                                     bias=nmax, scale=1.0, accum_out=rsum)
                rg = smalls.tile([P, 1], f32, tag="rg", name="rg")
                nc.vector.reciprocal(out=rg, in_=rsum)
                nc.vector.tensor_mul(out=rg, in0=rg, in1=gamma_sb)
                nc.vector.tensor_scalar_mul(out=exp_sb, in0=exp_sb, scalar1=rg)
                exps.append(exp_sb)
            for fn in cover:
                fn()
            for m in range(KC):
                for j in range(KC):
                    pst = psB.tile([P, P], f32r, tag="b", name="ps_tr")
                    nc.tensor.transpose(pst, exps[m][:, j * P:(j + 1) * P], ident)
                    nc.vector.tensor_copy(out=attnT_sb[:, j, m * P:(m + 1) * P],
                                          in_=pst)

        def ph4_chunk(st, s, cc):
            # y = gamma*attn@kv + img, overwriting img in place
            cs = cc * ch
            for m in range(KC):
                ps = psA.tile([P, ch], f32, tag="a", name="ps_ai")
                for j in range(KC):
                    nc.tensor.matmul(ps, lhsT=st["attnT"][:, j, m * P:(m + 1) * P],
                                     rhs=st["kvch"][cc][:, j, :],
                                     start=(j == 0), stop=(j == KC - 1))
                nc.vector.tensor_add(out=st["img"][:, m, cs:cs + ch], in0=ps,
                                     in1=st["img"][:, m, cs:cs + ch])
            st["kvch"][cc] = None

        def ph5_chunk(st, s, cc):
            cs = cc * ch
            for m2 in range(MO):
                ps = psA.tile([P, ch], f32, tag="a", name="ps_out")
                for j in range(KC):
                    nc.tensor.matmul(ps, lhsT=woT_sb[:, j, m2 * P:(m2 + 1) * P],
                                     rhs=st["img"][:, j, cs:cs + ch],
                                     start=(j == 0), stop=(j == KC - 1))
                ot = ostage.tile([P, ch], f32, tag="ot", name="ot")
                if m2 % 2 == 0:
                    nc.vector.tensor_copy(out=ot, in_=ps)
                else:
                    nc.scalar.copy(out=ot, in_=ps)
                nc.sync.dma_start(out=out[s, m2 * P:(m2 + 1) * P, cs:cs + ch],
                                  in_=ot)

        # ---- pipelined schedule: sample s-1's tail (last transposes, attn,
        # softmax, phases 4/5) is interleaved into sample s's pass-A chunks
        # so the PE never drains at sample boundaries.
        tails = []
        for s in range(spc):
            st = {"img": None, "kvch": [None] * NCH, "attn_ps": None,
                  "attnT": None, "imgT": [None] * NCH, "txtT": [None] * NCH}
            st["img"] = imgp.tile([P, KC, hw], f32r, tag="img", name=f"img{s}")
            for cc in range(NCH):
                passA_chunk(st, s, cc, first=(s == 0 and cc == 0))
                if s == 0 and cc == 0:
                    emit_deferred_constants()
                if cc >= 1:
                    transposes(st, s, cc - 1)
                if cc >= 2:
                    attn_chunk(st, s, cc - 2)
                npop = (3, 3, 3, 3, 2, 2, 1, 1)[min(cc, 7)]
                for _ in range(npop):
                    if tails:
                        tails.pop(0)()
            if s == spc - 1:
                # the final sample's softmax has no later pass-A to hide
                # behind; cover it with whatever of the previous sample's
                # tail is still pending (its last output-conv chunks).
                leftovers = tails[:]
                tails.clear()
                tails.extend([
                    (lambda st=st, s=s: transposes(st, s, NCH - 1)),
                    (lambda st=st, s=s: attn_chunk(st, s, NCH - 2)),
                    (lambda st=st, s=s: attn_chunk(st, s, NCH - 1)),
                    (lambda st=st, s=s, cov=tuple(leftovers):
                        softmax(st, s, cover=cov)),
                ])
            else:
                tails.extend([
                    (lambda st=st, s=s: transposes(st, s, NCH - 1)),
                    (lambda st=st, s=s: attn_chunk(st, s, NCH - 2)),
                    (lambda st=st, s=s: attn_chunk(st, s, NCH - 1)),
                    (lambda st=st, s=s: softmax(st, s)),
                ])
            tails.extend([(lambda st=st, s=s, cc=cc: ph4_chunk(st, s, cc))
                          for cc in range(NCH)])
            tails.extend([(lambda st=st, s=s, cc=cc: ph5_chunk(st, s, cc))
                          for cc in range(NCH)])
        while tails:
            tails.pop(0)()

    nc.compile()
    return nc


def _get_nc():
    key = "full"
    if key not in _BUILD_CACHE:
        _BUILD_CACHE[key] = _build_nc()
    return _BUILD_CACHE[key]


LAST_RESULTS = None  # BassKernelResults of the most recent kernel() call


def kernel(x1, x2, w_img, w_txt, w_out, gamma):
    import os
    from concourse.bass_utils import run_bass_kernel_spmd

    x1 = np.ascontiguousarray(np.asarray(x1, dtype=np.float32)).reshape(B, C1, HW)
    x2 = np.ascontiguousarray(np.asarray(x2, dtype=np.float32)).reshape(B, C2, HW)
    w_img = np.asarray(w_img, dtype=np.float32)
    w_txt = np.asarray(w_txt, dtype=np.float32)
    w_out = np.asarray(w_out, dtype=np.float32)
    gamma = np.ascontiguousarray(np.asarray(gamma, dtype=np.float32)).reshape(1)

    # pad x2 channels 320 -> 384 with zeros so every k-tile is 128 deep
    x2p = np.zeros((B, C2P, HW), dtype=np.float32)
    x2p[:, :C2, :] = x2

    w_imgT = np.ascontiguousarray(w_img.T)              # [512, 256]
    w_txtT = np.zeros((C2P, C), dtype=np.float32)       # [384, 256]
    w_txtT[:C2, :] = w_txt.T
    w_outT = np.ascontiguousarray(w_out.T)              # [256, 512]

    nc = _get_nc()
    ident = np.eye(128, dtype=np.float32)
    in_maps = []
    for core in range(NCORES):
        s0 = core * SPC
        in_maps.append({
            "x1": np.ascontiguousarray(x1[s0:s0 + SPC]),
            "x2": np.ascontiguousarray(x2p[s0:s0 + SPC]),
            "w_imgT": w_imgT,
            "w_txtT": w_txtT,
            "w_outT": w_outT,
            "gamma": gamma,
            "ident": ident,
            "nonce": np.zeros((1, _nonce_len()), dtype=np.float32),
        })

    kwargs = {}
    if os.environ.get("KERNEL_TRACE"):
        kwargs["trace"] = True
        if os.environ.get("KERNEL_TRACE_DIR"):
            kwargs["tmpdir"] = os.environ["KERNEL_TRACE_DIR"]
    res = run_bass_kernel_spmd(nc, in_maps, core_ids=list(range(NCORES)), **kwargs)
    global LAST_RESULTS
    LAST_RESULTS = res
    outs = [res.results[c]["out"] for c in range(NCORES)]
    full = np.concatenate(outs, axis=0).reshape(B, C1, 64, 64)
    return full


if __name__ == "__main__":
    rng = np.random.default_rng(0)
    inputs = {
        "x1": rng.standard_normal((B, C1, 64, 64), dtype=np.float32),
        "x2": rng.standard_normal((B, C2, 64, 64), dtype=np.float32),
        "w_img": rng.standard_normal((C, C1), dtype=np.float32) / np.sqrt(C1),
        "w_txt": rng.standard_normal((C, C2), dtype=np.float32) / np.sqrt(C2),
        "w_out": rng.standard_normal((C1, C), dtype=np.float32) / np.sqrt(C),
        "gamma": rng.standard_normal(1).astype(np.float32),
    }
    out = kernel(**inputs)
    print(out.shape, out.dtype)


# revision 36
# speedup vs baseline: 1.0191x; 1.0191x over previous
"""Trainium2 Bass kernel for the CAM (cross-attention module) problem.

Math (per sample b):
    img = w_img @ x1_b          # [256, 4096]
    kv  = w_txt @ x2_b          # [256, 4096]
    attn = softmax(img @ kv^T)  # [256, 256], softmax over last dim
    y = gamma * (attn @ kv) + img
    out_b = w_out @ y           # [512, 4096]

Sharding: data-parallel over batch, 16 samples -> 2 per core x 8 cores,
no collectives.

Precision: all matmuls run in float32r (TRN2's single-pass fp32 matmul
mode, ~1.5e-4 per-product vs 1e-7 for the 2x-slower LOW_HIGH fp32 mode).
End-to-end relative error vs the fp32 reference is ~1.3e-3.

Layout strategy: the spatial contraction (attn logits) needs
spatial-major operands while the residual + output conv need
channel-major ones.  img/kv are computed channel-major from the natural
HBM layout, and the spatial-major copies are made with PE transposes
(one 128x128 transpose per block - far cheaper than re-contracting over
input channels).  The two samples per core are software-pipelined: each
sample's tail (softmax + attn@kv + output conv) is interleaved into the
next sample's projection chunks so the tensor engine never drains.
"""

import numpy as np

# Problem shapes (hardcoded per the harness contract)
B = 16
C1 = 512          # x1 channels (also output channels)
C2 = 320          # x2 channels
C2P = 384         # x2 channels padded to a multiple of 128 (K<128 matmuls
                  # and partition-offset memsets are both broken on HW)
C = 256           # projected channels
HW = 64 * 64      # spatial size
NCORES = 8
SPC = B // NCORES  # samples per core

_BUILD_CACHE = {}


def _nonce_len():
    import inspect
    import zlib
    return 2 + (zlib.crc32(inspect.getsource(_build_nc).encode()) % 997)


def _build_nc(spc=SPC, c1=C1, c2p=C2P, c=C, hw=HW, ch=512):
    """Build the per-core Bass program (same program on all cores)."""
    import concourse.tile as tile
    from concourse import bacc, mybir

    f32 = mybir.dt.float32
    f32r = mybir.dt.float32r
    P = 128
    K1 = c1 // P           # k-tiles of x1 channels
    K2 = c2p // P          # k-tiles of x2 channels (padded)
    KC = c // P            # k-tiles of projected channels
    MO = c1 // P           # m-tiles of output conv
    NCH = hw // ch         # spatial chunks
    TPC = ch // P          # 128-wide spatial tiles per chunk

    # Bacc (not plain Bass): its compile() runs move_matmul_waits_to_ldweights
    # + generate_event_semaphores, without which walrus rejects any Matmult
    # carrying more than one semaphore wait.
    nc = bacc.Bacc("TRN2", target_bir_lowering=False)
    x1 = nc.declare_dram_parameter("x1", [spc, c1, hw], f32r, isOutput=False)
    x2 = nc.declare_dram_parameter("x2", [spc, c2p, hw], f32r, isOutput=False)
    wiT = nc.declare_dram_parameter("w_imgT", [c1, c], f32r, isOutput=False)
    wtT = nc.declare_dram_parameter("w_txtT", [c2p, c], f32r, isOutput=False)
    woT = nc.declare_dram_parameter("w_outT", [c, c1], f32r, isOutput=False)
    gamma = nc.declare_dram_parameter("gamma", [1], f32, isOutput=False)
    idin = nc.declare_dram_parameter("ident", [P, P], f32r, isOutput=False)
    # The PJRT executable cache fingerprints the HLO without the embedded
    # BIR payload, so two different kernels with identical I/O signatures
    # collide. A source-hash-sized dummy input makes the signature unique.
    nc.declare_dram_parameter("nonce", [1, _nonce_len()], f32, isOutput=False)
    out = nc.declare_dram_parameter("out", [spc, c1, hw], f32, isOutput=True)

    Exp = mybir.ActivationFunctionType.Exp
    X = mybir.AxisListType.X

    with (
        tile.TileContext(nc) as tc,
        tc.tile_pool(name="singles", bufs=1) as singles,
        tc.tile_pool(name="xin", bufs=2) as xin,
        tc.tile_pool(name="tch", bufs=2) as tch,
        tc.tile_pool(name="imgp", bufs=2) as imgp,
        tc.tile_pool(name="kvp", bufs=NCH + 2) as kvp,
        tc.tile_pool(name="attnsb", bufs=2) as attnsb,
        tc.tile_pool(name="smalls", bufs=4) as smalls,
        tc.tile_pool(name="ostage", bufs=4) as ostage,
        tc.tile_pool(name="psA", bufs=4, space="PSUM") as psA,
        tc.tile_pool(name="psB", bufs=2, space="PSUM") as psB,
        tc.tile_pool(name="psAttn", bufs=2, space="PSUM") as psAttn,
    ):
        # ---- constants. Weight loads for chunk 0 are emitted interleaved
        # with the chunk-0 x loads (see passA_chunk) so the SP queue issues
        # the first matmul's inputs as early as possible; woT/ident/gamma
        # are deferred past the first chunk since nothing needs them early.
        wiT_sb = singles.tile([P, K1, c], f32r)
        wtT_sb = singles.tile([P, K2, c], f32r)
        woT_sb = singles.tile([P, KC, c1], f32r)
        ident = singles.tile([P, P], f32r)
        gamma_sb = singles.tile([P, 1], f32)

        def emit_deferred_constants():
            for k in range(KC):
                nc.sync.dma_start(out=woT_sb[:, k, :], in_=woT[k * P:(k + 1) * P, :])
            nc.sync.dma_start(out=ident, in_=idin[:])
            nc.sync.dma_start(out=gamma_sb, in_=gamma[:].to_broadcast((P, 1)))

        # ---- per-sample emission helpers -------------------------------
        def passA_chunk(st, s, cc, first=False):
            # `first`: interleave the weight loads with the chunk-0 x loads
            # so the SP queue issues the first matmul's inputs earliest.
            cs = cc * ch
            x1c = xin.tile([P, K1, ch], f32r, tag="x1c", name="x1c", bufs=3)
            for k in range(K1):
                nc.sync.dma_start(out=x1c[:, k, :],
                                  in_=x1[s, k * P:(k + 1) * P, cs:cs + ch])
                if first:
                    nc.sync.dma_start(out=wiT_sb[:, k, :],
                                      in_=wiT[k * P:(k + 1) * P, :])
            x2c = xin.tile([P, K2, ch], f32r, tag="x2c", name="x2c")
            for k in range(K2):
                kp = min(c2p - k * P, P)
                nc.sync.dma_start(out=x2c[:kp, k, :],
                                  in_=x2[s, k * P:k * P + kp, cs:cs + ch])
                if first:
                    nc.sync.dma_start(out=wtT_sb[:kp, k, :],
                                      in_=wtT[k * P:k * P + kp, :])
            for m in range(KC):
                ps = psA.tile([P, ch], f32, tag="a", name="ps_img")
                for k in range(K1):
                    nc.tensor.matmul(ps, lhsT=wiT_sb[:, k, m * P:(m + 1) * P],
                                     rhs=x1c[:, k, :],
                                     start=(k == 0), stop=(k == K1 - 1))
                nc.vector.tensor_copy(out=st["img"][:, m, cs:cs + ch], in_=ps)
            kvt = kvp.tile([P, KC, ch], f32r, tag="kv", name="kvt")
            st["kvch"][cc] = kvt
            for m in range(KC):
                ps = psA.tile([P, ch], f32, tag="a", name="ps_kv")
                for k in range(K2):
                    nc.tensor.matmul(ps, lhsT=wtT_sb[:, k, m * P:(m + 1) * P],
                                     rhs=x2c[:, k, :],
                                     start=(k == 0), stop=(k == K2 - 1))
                nc.vector.tensor_copy(out=kvt[:, m, :], in_=ps)

        def transposes(st, s, pc):
            # spatial-major orientations via PE transpose of img / kv chunks
            pcs = pc * ch
            imgT_c = tch.tile([P, TPC, c], f32r, tag="imgT", name="imgT_c")
            for t in range(TPC):
                ps = psB.tile([P, c], f32r, tag="b", name="ps_imgT")
                for i in range(KC):
                    nc.tensor.transpose(
                        ps[:, i * P:(i + 1) * P],
                        st["img"][:, i, pcs + t * P:pcs + (t + 1) * P], ident)
                nc.scalar.copy(out=imgT_c[:, t, :], in_=ps)
            txtT_c = tch.tile([P, TPC, c], f32r, tag="txtT", name="txtT_c")
            for t in range(TPC):
                ps = psB.tile([P, c], f32r, tag="b", name="ps_txtT")
                for i in range(KC):
                    nc.tensor.transpose(
                        ps[:, i * P:(i + 1) * P],
                        st["kvch"][pc][:, i, t * P:(t + 1) * P], ident)
                nc.scalar.copy(out=txtT_c[:, t, :], in_=ps)
            st["imgT"][pc] = imgT_c
            st["txtT"][pc] = txtT_c

        def attn_chunk(st, s, pc):
            if st["attn_ps"] is None:
                st["attn_ps"] = [
                    psAttn.tile([P, c], f32, tag="attn", name=f"attn{s}_{m}")
                    for m in range(KC)
                ]
            for m in range(KC):
                for t in range(TPC):
                    nc.tensor.matmul(
                        st["attn_ps"][m],
                        lhsT=st["imgT"][pc][:, t, m * P:(m + 1) * P],
                        rhs=st["txtT"][pc][:, t, :],
                        start=(pc == 0 and t == 0),
                        stop=(pc == NCH - 1 and t == TPC - 1))
            st["imgT"][pc] = st["txtT"][pc] = None

        def softmax(st, s, cover=()):
            # softmax over the free (d) axis, gamma folded in; transpose to
            # attnT [d, c] for the attn@kv contraction.  `cover` closures are
            # emitted between the DVE/ACT stats and the PE transposes so the
            # tensor engine has work while the serial softmax chain runs.
            attnT_sb = attnsb.tile([P, KC, c], f32r, tag="attnT", name="attnT")
            st["attnT"] = attnT_sb
            exps = []
            for m in range(KC):
                nmax = smalls.tile([P, 1], f32, tag="nmax", name="nmax")
                nc.vector.reduce_max(out=nmax, in_=st["attn_ps"][m], axis=X,
                                     negate=True)
                exp_sb = smalls.tile([P, c], f32r, tag="exp", name="exp_sb")
                rsum = smalls.tile([P, 1], f32, tag="rsum", name="rsum")
                nc.scalar.activation(out=exp_sb, in_=st["attn_ps"][m], func=Exp,
                                     bias=nmax, scale=1.0, accum_out=rsum)
                rg = smalls.tile([P, 1], f32, tag="rg", name="rg")
                nc.vector.reciprocal(out=rg, in_=rsum)
                nc.vector.tensor_mul(out=rg, in0=rg, in1=gamma_sb)
                nc.vector.tensor_scalar_mul(out=exp_sb, in0=exp_sb, scalar1=rg)
                exps.append(exp_sb)
            for fn in cover:
                fn()
            for m in range(KC):
                for j in range(KC):
                    pst = psB.tile([P, P], f32r, tag="b", name="ps_tr")
                    nc.tensor.transpose(pst, exps[m][:, j * P:(j + 1) * P], ident)
                    nc.vector.tensor_copy(out=attnT_sb[:, j, m * P:(m + 1) * P],
                                          in_=pst)

        def ph4_chunk(st, s, cc):
            # y = gamma*attn@kv + img, overwriting img in place
            cs = cc * ch
            for m in range(KC):
                ps = psA.tile([P, ch], f32, tag="a", name="ps_ai")
                for j in range(KC):
                    nc.tensor.matmul(ps, lhsT=st["attnT"][:, j, m * P:(m + 1) * P],
                                     rhs=st["kvch"][cc][:, j, :],
                                     start=(j == 0), stop=(j == KC - 1))
                nc.vector.tensor_add(out=st["img"][:, m, cs:cs + ch], in0=ps,
                                     in1=st["img"][:, m, cs:cs + ch])
            st["kvch"][cc] = None

        def ph5_chunk(st, s, cc):
            cs = cc * ch
            for m2 in range(MO):
                ps = psA.tile([P, ch], f32, tag="a", name="ps_out")
                for j in range(KC):
                    nc.tensor.matmul(ps, lhsT=woT_sb[:, j, m2 * P:(m2 + 1) * P],
                                     rhs=st["img"][:, j, cs:cs + ch],
                                     start=(j == 0), stop=(j == KC - 1))
                ot = ostage.tile([P, ch], f32, tag="ot", name="ot")
                if m2 % 2 == 0:
                    nc.vector.tensor_copy(out=ot, in_=ps)
                else:
                    nc.scalar.copy(out=ot, in_=ps)
                nc.sync.dma_start(out=out[s, m2 * P:(m2 + 1) * P, cs:cs + ch],
                                  in_=ot)

        # ---- pipelined schedule: sample s-1's tail (last transposes, attn,
        # softmax, phases 4/5) is interleaved into sample s's pass-A chunks
        # so the PE never drains at sample boundaries.
        tails = []
        for s in range(spc):
            st = {"img": None, "kvch": [None] * NCH, "attn_ps": None,
                  "attnT": None, "imgT": [None] * NCH, "txtT": [None] * NCH}
            st["img"] = imgp.tile([P, KC, hw], f32r, tag="img", name=f"img{s}")
            for cc in range(NCH):
                passA_chunk(st, s, cc, first=(s == 0 and cc == 0))
                if s == 0 and cc == 0:
                    emit_deferred_constants()
                if cc >= 1:
                    transposes(st, s, cc - 1)
                if cc >= 2:
                    attn_chunk(st, s, cc - 2)
                npop = (3, 3, 3, 3, 2, 2, 1, 1)[min(cc, 7)]
                for _ in range(npop):
                    if tails:
                        tails.pop(0)()
            if s == spc - 1:
                # the final sample's softmax has no later pass-A to hide
                # behind; cover it with whatever of the previous sample's
                # tail is still pending (its last output-conv chunks).
                leftovers = tails[:]
                tails.clear()
                tails.extend([
                    (lambda st=st, s=s: transposes(st, s, NCH - 1)),
                    (lambda st=st, s=s: attn_chunk(st, s, NCH - 2)),
                    (lambda st=st, s=s: attn_chunk(st, s, NCH - 1)),
                    (lambda st=st, s=s, cov=tuple(leftovers):
                        softmax(st, s, cover=cov)),
                ])
            else:
                tails.extend([
                    (lambda st=st, s=s: transposes(st, s, NCH - 1)),
                    (lambda st=st, s=s: attn_chunk(st, s, NCH - 2)),
                    (lambda st=st, s=s: attn_chunk(st, s, NCH - 1)),
                    (lambda st=st, s=s: softmax(st, s)),
                ])
            tails.extend([(lambda st=st, s=s, cc=cc: ph4_chunk(st, s, cc))
                          for cc in range(NCH)])
            tails.extend([(lambda st=st, s=s, cc=cc: ph5_chunk(st, s, cc))
                          for cc in range(NCH)])
        while tails:
            tails.pop(0)()

    nc.compile()
    return nc


def _get_nc():
    key = "full"
    if key not in _BUILD_CACHE:
        _BUILD_CACHE[key] = _build_nc()
    return _BUILD_CACHE[key]


LAST_RESULTS = None  # BassKernelResults of the most recent kernel() call


def kernel(x1, x2, w_img, w_txt, w_out, gamma):
    import os
    from concourse.bass_utils import run_bass_kernel_spmd

    x1 = np.ascontiguousarray(np.asarray(x1, dtype=np.float32)).reshape(B, C1, HW)
    x2 = np.ascontiguousarray(np.asarray(x2, dtype=np.float32)).reshape(B, C2, HW)
    w_img = np.asarray(w_img, dtype=np.float32)
    w_txt = np.asarray(w_txt, dtype=np.float32)
    w_out = np.asarray(w_out, dtype=np.float32)
    gamma = np.ascontiguousarray(np.asarray(gamma, dtype=np.float32)).reshape(1)

    # pad x2 channels 320 -> 384 with zeros so every k-tile is 128 deep
    x2p = np.zeros((B, C2P, HW), dtype=np.float32)
    x2p[:, :C2, :] = x2

    w_imgT = np.ascontiguousarray(w_img.T)              # [512, 256]
    w_txtT = np.zeros((C2P, C), dtype=np.float32)       # [384, 256]
    w_txtT[:C2, :] = w_txt.T
    w_outT = np.ascontiguousarray(w_out.T)              # [256, 512]

    nc = _get_nc()
    ident = np.eye(128, dtype=np.float32)
    in_maps = []
    for core in range(NCORES):
        s0 = core * SPC
        in_maps.append({
            "x1": np.ascontiguousarray(x1[s0:s0 + SPC]),
            "x2": np.ascontiguousarray(x2p[s0:s0 + SPC]),
            "w_imgT": w_imgT,
            "w_txtT": w_txtT,
            "w_outT": w_outT,
            "gamma": gamma,
            "ident": ident,
            "nonce": np.zeros((1, _nonce_len()), dtype=np.float32),
        })

    kwargs = {}
    if os.environ.get("KERNEL_TRACE"):
        kwargs["trace"] = True
        if os.environ.get("KERNEL_TRACE_DIR"):
            kwargs["tmpdir"] = os.environ["KERNEL_TRACE_DIR"]
    res = run_bass_kernel_spmd(nc, in_maps, core_ids=list(range(NCORES)), **kwargs)
    global LAST_RESULTS
    LAST_RESULTS = res
    outs = [res.results[c]["out"] for c in range(NCORES)]
    full = np.concatenate(outs, axis=0).reshape(B, C1, 64, 64)
    return full


if __name__ == "__main__":
    rng = np.random.default_rng(0)
    inputs = {
        "x1": rng.standard_normal((B, C1, 64, 64), dtype=np.float32),
        "x2": rng.standard_normal((B, C2, 64, 64), dtype=np.float32),
        "w_img": rng.standard_normal((C, C1), dtype=np.float32) / np.sqrt(C1),
        "w_txt": rng.standard_normal((C, C2), dtype=np.float32) / np.sqrt(C2),
        "w_out": rng.standard_normal((C1, C), dtype=np.float32) / np.sqrt(C),
        "gamma": rng.standard_normal(1).astype(np.float32),
    }
    out = kernel(**inputs)
    print(out.shape, out.dtype)


# revision 37
# speedup vs baseline: 1.0424x; 1.0229x over previous
"""Trainium2 Bass kernel for the CAM (cross-attention module) problem.

Math (per sample b):
    img = w_img @ x1_b          # [256, 4096]
    kv  = w_txt @ x2_b          # [256, 4096]
    attn = softmax(img @ kv^T)  # [256, 256], softmax over last dim
    y = gamma * (attn @ kv) + img
    out_b = w_out @ y           # [512, 4096]

Sharding: data-parallel over batch, 16 samples -> 2 per core x 8 cores,
no collectives.

Precision: all matmuls run in float32r (TRN2's single-pass fp32 matmul
mode, ~1.5e-4 per-product vs 1e-7 for the 2x-slower LOW_HIGH fp32 mode).
End-to-end relative error vs the fp32 reference is ~1.3e-3.

Layout strategy: the spatial contraction (attn logits) needs
spatial-major operands while the residual + output conv need
channel-major ones.  img/kv are computed channel-major from the natural
HBM layout, and the spatial-major copies are made with PE transposes
(one 128x128 transpose per block - far cheaper than re-contracting over
input channels).  The two samples per core are software-pipelined: each
sample's tail (softmax + attn@kv + output conv) is interleaved into the
next sample's projection chunks so the tensor engine never drains.
"""

import numpy as np

# Problem shapes (hardcoded per the harness contract)
B = 16
C1 = 512          # x1 channels (also output channels)
C2 = 320          # x2 channels
C2P = 384         # x2 channels padded to a multiple of 128 (K<128 matmuls
                  # and partition-offset memsets are both broken on HW)
C = 256           # projected channels
HW = 64 * 64      # spatial size
NCORES = 8
SPC = B // NCORES  # samples per core

_BUILD_CACHE = {}


def _nonce_len():
    import inspect
    import zlib
    return 2 + (zlib.crc32(inspect.getsource(_build_nc).encode()) % 997)


def _build_nc(spc=SPC, c1=C1, c2p=C2P, c=C, hw=HW, ch=512):
    """Build the per-core Bass program (same program on all cores)."""
    import concourse.tile as tile
    from concourse import bacc, mybir

    f32 = mybir.dt.float32
    f32r = mybir.dt.float32r
    P = 128
    K1 = c1 // P           # k-tiles of x1 channels
    K2 = c2p // P          # k-tiles of x2 channels (padded)
    KC = c // P            # k-tiles of projected channels
    MO = c1 // P           # m-tiles of output conv
    NCH = hw // ch         # spatial chunks
    TPC = ch // P          # 128-wide spatial tiles per chunk

    # Bacc (not plain Bass): its compile() runs move_matmul_waits_to_ldweights
    # + generate_event_semaphores, without which walrus rejects any Matmult
    # carrying more than one semaphore wait.
    nc = bacc.Bacc("TRN2", target_bir_lowering=False)
    x1 = nc.declare_dram_parameter("x1", [spc, c1, hw], f32r, isOutput=False)
    x2 = nc.declare_dram_parameter("x2", [spc, c2p, hw], f32r, isOutput=False)
    wiT = nc.declare_dram_parameter("w_imgT", [c1, c], f32r, isOutput=False)
    wtT = nc.declare_dram_parameter("w_txtT", [c2p, c], f32r, isOutput=False)
    woT = nc.declare_dram_parameter("w_outT", [c, c1], f32r, isOutput=False)
    gamma = nc.declare_dram_parameter("gamma", [1], f32, isOutput=False)
    idin = nc.declare_dram_parameter("ident", [P, P], f32r, isOutput=False)
    # The PJRT executable cache fingerprints the HLO without the embedded
    # BIR payload, so two different kernels with identical I/O signatures
    # collide. A source-hash-sized dummy input makes the signature unique.
    nc.declare_dram_parameter("nonce", [1, _nonce_len()], f32, isOutput=False)
    out = nc.declare_dram_parameter("out", [spc, c1, hw], f32, isOutput=True)

    Exp = mybir.ActivationFunctionType.Exp
    X = mybir.AxisListType.X

    with (
        tile.TileContext(nc) as tc,
        tc.tile_pool(name="singles", bufs=1) as singles,
        tc.tile_pool(name="xin", bufs=2) as xin,
        tc.tile_pool(name="tch", bufs=2) as tch,
        tc.tile_pool(name="imgp", bufs=2) as imgp,
        tc.tile_pool(name="kvp", bufs=NCH + 2) as kvp,
        tc.tile_pool(name="attnsb", bufs=2) as attnsb,
        tc.tile_pool(name="smalls", bufs=4) as smalls,
        tc.tile_pool(name="ostage", bufs=6) as ostage,
        tc.tile_pool(name="psA", bufs=4, space="PSUM") as psA,
        tc.tile_pool(name="psB", bufs=2, space="PSUM") as psB,
        tc.tile_pool(name="psAttn", bufs=2, space="PSUM") as psAttn,
    ):
        # ---- constants. Weight loads for chunk 0 are emitted interleaved
        # with the chunk-0 x loads (see passA_chunk) so the SP queue issues
        # the first matmul's inputs as early as possible; woT/ident/gamma
        # are deferred past the first chunk since nothing needs them early.
        wiT_sb = singles.tile([P, K1, c], f32r)
        wtT_sb = singles.tile([P, K2, c], f32r)
        woT_sb = singles.tile([P, KC, c1], f32r)
        ident = singles.tile([P, P], f32r)
        gamma_sb = singles.tile([P, 1], f32)

        def emit_deferred_constants():
            for k in range(KC):
                nc.sync.dma_start(out=woT_sb[:, k, :], in_=woT[k * P:(k + 1) * P, :])
            nc.sync.dma_start(out=ident, in_=idin[:])
            nc.sync.dma_start(out=gamma_sb, in_=gamma[:].to_broadcast((P, 1)))

        # ---- per-sample emission helpers -------------------------------
        def passA_chunk(st, s, cc, first=False):
            # `first`: interleave the weight loads with the chunk-0 x loads
            # so the SP queue issues the first matmul's inputs earliest.
            cs = cc * ch
            x1c = xin.tile([P, K1, ch], f32r, tag="x1c", name="x1c", bufs=3)
            for k in range(K1):
                nc.sync.dma_start(out=x1c[:, k, :],
                                  in_=x1[s, k * P:(k + 1) * P, cs:cs + ch])
                if first:
                    nc.sync.dma_start(out=wiT_sb[:, k, :],
                                      in_=wiT[k * P:(k + 1) * P, :])
            x2c = xin.tile([P, K2, ch], f32r, tag="x2c", name="x2c")
            for k in range(K2):
                kp = min(c2p - k * P, P)
                nc.sync.dma_start(out=x2c[:kp, k, :],
                                  in_=x2[s, k * P:k * P + kp, cs:cs + ch])
                if first:
                    nc.sync.dma_start(out=wtT_sb[:kp, k, :],
                                      in_=wtT[k * P:k * P + kp, :])
            for m in range(KC):
                ps = psA.tile([P, ch], f32, tag="a", name="ps_img")
                for k in range(K1):
                    nc.tensor.matmul(ps, lhsT=wiT_sb[:, k, m * P:(m + 1) * P],
                                     rhs=x1c[:, k, :],
                                     start=(k == 0), stop=(k == K1 - 1))
                nc.vector.tensor_copy(out=st["img"][:, m, cs:cs + ch], in_=ps)
            kvt = kvp.tile([P, KC, ch], f32r, tag="kv", name="kvt")
            st["kvch"][cc] = kvt
            for m in range(KC):
                ps = psA.tile([P, ch], f32, tag="a", name="ps_kv")
                for k in range(K2):
                    nc.tensor.matmul(ps, lhsT=wtT_sb[:, k, m * P:(m + 1) * P],
                                     rhs=x2c[:, k, :],
                                     start=(k == 0), stop=(k == K2 - 1))
                nc.vector.tensor_copy(out=kvt[:, m, :], in_=ps)

        def transposes(st, s, pc):
            # spatial-major orientations via PE transpose of img / kv chunks
            pcs = pc * ch
            imgT_c = tch.tile([P, TPC, c], f32r, tag="imgT", name="imgT_c")
            for t in range(TPC):
                ps = psB.tile([P, c], f32r, tag="b", name="ps_imgT")
                for i in range(KC):
                    nc.tensor.transpose(
                        ps[:, i * P:(i + 1) * P],
                        st["img"][:, i, pcs + t * P:pcs + (t + 1) * P], ident)
                nc.scalar.copy(out=imgT_c[:, t, :], in_=ps)
            txtT_c = tch.tile([P, TPC, c], f32r, tag="txtT", name="txtT_c")
            for t in range(TPC):
                ps = psB.tile([P, c], f32r, tag="b", name="ps_txtT")
                for i in range(KC):
                    nc.tensor.transpose(
                        ps[:, i * P:(i + 1) * P],
                        st["kvch"][pc][:, i, t * P:(t + 1) * P], ident)
                nc.scalar.copy(out=txtT_c[:, t, :], in_=ps)
            st["imgT"][pc] = imgT_c
            st["txtT"][pc] = txtT_c

        def attn_chunk(st, s, pc):
            if st["attn_ps"] is None:
                st["attn_ps"] = [
                    psAttn.tile([P, c], f32, tag="attn", name=f"attn{s}_{m}")
                    for m in range(KC)
                ]
            for m in range(KC):
                for t in range(TPC):
                    nc.tensor.matmul(
                        st["attn_ps"][m],
                        lhsT=st["imgT"][pc][:, t, m * P:(m + 1) * P],
                        rhs=st["txtT"][pc][:, t, :],
                        start=(pc == 0 and t == 0),
                        stop=(pc == NCH - 1 and t == TPC - 1))
            st["imgT"][pc] = st["txtT"][pc] = None

        def softmax(st, s, cover=()):
            # softmax over the free (d) axis, gamma folded in; transpose to
            # attnT [d, c] for the attn@kv contraction.  `cover` closures are
            # emitted between the DVE/ACT stats and the PE transposes so the
            # tensor engine has work while the serial softmax chain runs.
            attnT_sb = attnsb.tile([P, KC, c], f32r, tag="attnT", name="attnT")
            st["attnT"] = attnT_sb
            exps = []
            for m in range(KC):
                nmax = smalls.tile([P, 1], f32, tag="nmax", name="nmax")
                nc.vector.reduce_max(out=nmax, in_=st["attn_ps"][m], axis=X,
                                     negate=True)
                exp_sb = smalls.tile([P, c], f32r, tag="exp", name="exp_sb")
                rsum = smalls.tile([P, 1], f32, tag="rsum", name="rsum")
                nc.scalar.activation(out=exp_sb, in_=st["attn_ps"][m], func=Exp,
                                     bias=nmax, scale=1.0, accum_out=rsum)
                rg = smalls.tile([P, 1], f32, tag="rg", name="rg")
                nc.vector.reciprocal(out=rg, in_=rsum)
                nc.vector.tensor_mul(out=rg, in0=rg, in1=gamma_sb)
                nc.vector.tensor_scalar_mul(out=exp_sb, in0=exp_sb, scalar1=rg)
                exps.append(exp_sb)
            for fn in cover:
                fn()
            for m in range(KC):
                for j in range(KC):
                    pst = psB.tile([P, P], f32r, tag="b", name="ps_tr")
                    nc.tensor.transpose(pst, exps[m][:, j * P:(j + 1) * P], ident)
                    nc.vector.tensor_copy(out=attnT_sb[:, j, m * P:(m + 1) * P],
                                          in_=pst)

        def ph4_chunk(st, s, cc):
            # y = gamma*attn@kv + img, overwriting img in place
            cs = cc * ch
            for m in range(KC):
                ps = psA.tile([P, ch], f32, tag="a", name="ps_ai")
                for j in range(KC):
                    nc.tensor.matmul(ps, lhsT=st["attnT"][:, j, m * P:(m + 1) * P],
                                     rhs=st["kvch"][cc][:, j, :],
                                     start=(j == 0), stop=(j == KC - 1))
                nc.vector.tensor_add(out=st["img"][:, m, cs:cs + ch], in0=ps,
                                     in1=st["img"][:, m, cs:cs + ch])
            st["kvch"][cc] = None

        def ph5_chunk(st, s, cc):
            cs = cc * ch
            for m2 in range(MO):
                ps = psA.tile([P, ch], f32, tag="a", name="ps_out")
                for j in range(KC):
                    nc.tensor.matmul(ps, lhsT=woT_sb[:, j, m2 * P:(m2 + 1) * P],
                                     rhs=st["img"][:, j, cs:cs + ch],
                                     start=(j == 0), stop=(j == KC - 1))
                ot = ostage.tile([P, ch], f32, tag="ot", name="ot")
                if m2 % 2 == 0:
                    nc.vector.tensor_copy(out=ot, in_=ps)
                else:
                    nc.scalar.copy(out=ot, in_=ps)
                nc.sync.dma_start(out=out[s, m2 * P:(m2 + 1) * P, cs:cs + ch],
                                  in_=ot)

        # ---- pipelined schedule: sample s-1's tail (last transposes, attn,
        # softmax, phases 4/5) is interleaved into sample s's pass-A chunks
        # so the PE never drains at sample boundaries.
        tails = []
        for s in range(spc):
            st = {"img": None, "kvch": [None] * NCH, "attn_ps": None,
                  "attnT": None, "imgT": [None] * NCH, "txtT": [None] * NCH}
            st["img"] = imgp.tile([P, KC, hw], f32r, tag="img", name=f"img{s}")
            for cc in range(NCH):
                passA_chunk(st, s, cc, first=(s == 0 and cc == 0))
                if s == 0 and cc == 0:
                    emit_deferred_constants()
                if cc >= 1:
                    transposes(st, s, cc - 1)
                if cc >= 2:
                    attn_chunk(st, s, cc - 2)
                npop = (3, 3, 3, 3, 2, 2, 1, 1)[min(cc, 7)]
                for _ in range(npop):
                    if tails:
                        tails.pop(0)()
            if s == spc - 1:
                # the final sample's softmax has no later pass-A to hide
                # behind; cover it with whatever of the previous sample's
                # tail is still pending (its last output-conv chunks).
                leftovers = tails[:]
                tails.clear()
                tails.extend([
                    (lambda st=st, s=s: transposes(st, s, NCH - 1)),
                    (lambda st=st, s=s: attn_chunk(st, s, NCH - 2)),
                    (lambda st=st, s=s: attn_chunk(st, s, NCH - 1)),
                    (lambda st=st, s=s, cov=tuple(leftovers):
                        softmax(st, s, cover=cov)),
                ])
            else:
                tails.extend([
                    (lambda st=st, s=s: transposes(st, s, NCH - 1)),
                    (lambda st=st, s=s: attn_chunk(st, s, NCH - 2)),
                    (lambda st=st, s=s: attn_chunk(st, s, NCH - 1)),
                    (lambda st=st, s=s: softmax(st, s)),
                ])
            tails.extend([(lambda st=st, s=s, cc=cc: ph4_chunk(st, s, cc))
                          for cc in range(NCH)])
            tails.extend([(lambda st=st, s=s, cc=cc: ph5_chunk(st, s, cc))
                          for cc in range(NCH)])
        while tails:
            tails.pop(0)()

    nc.compile()
    return nc


def _get_nc():
    key = "full"
    if key not in _BUILD_CACHE:
        _BUILD_CACHE[key] = _build_nc()
    return _BUILD_CACHE[key]


LAST_RESULTS = None  # BassKernelResults of the most recent kernel() call


def kernel(x1, x2, w_img, w_txt, w_out, gamma):
    import os
    from concourse.bass_utils import run_bass_kernel_spmd

    x1 = np.ascontiguousarray(np.asarray(x1, dtype=np.float32)).reshape(B, C1, HW)
    x2 = np.ascontiguousarray(np.asarray(x2, dtype=np.float32)).reshape(B, C2, HW)
    w_img = np.asarray(w_img, dtype=np.float32)
    w_txt = np.asarray(w_txt, dtype=np.float32)
    w_out = np.asarray(w_out, dtype=np.float32)
    gamma = np.ascontiguousarray(np.asarray(gamma, dtype=np.float32)).reshape(1)

    # pad x2 channels 320 -> 384 with zeros so every k-tile is 128 deep
    x2p = np.zeros((B, C2P, HW), dtype=np.float32)
    x2p[:, :C2, :] = x2

    w_imgT = np.ascontiguousarray(w_img.T)              # [512, 256]
    w_txtT = np.zeros((C2P, C), dtype=np.float32)       # [384, 256]
    w_txtT[:C2, :] = w_txt.T
    w_outT = np.ascontiguousarray(w_out.T)              # [256, 512]

    nc = _get_nc()
    ident = np.eye(128, dtype=np.float32)
    in_maps = []
    for core in range(NCORES):
        s0 = core * SPC
        in_maps.append({
            "x1": np.ascontiguousarray(x1[s0:s0 + SPC]),
            "x2": np.ascontiguousarray(x2p[s0:s0 + SPC]),
            "w_imgT": w_imgT,
            "w_txtT": w_txtT,
            "w_outT": w_outT,
            "gamma": gamma,
            "ident": ident,
            "nonce": np.zeros((1, _nonce_len()), dtype=np.float32),
        })

    kwargs = {}
    if os.environ.get("KERNEL_TRACE"):
        kwargs["trace"] = True
        if os.environ.get("KERNEL_TRACE_DIR"):
            kwargs["tmpdir"] = os.environ["KERNEL_TRACE_DIR"]
    res = run_bass_kernel_spmd(nc, in_maps, core_ids=list(range(NCORES)), **kwargs)
    global LAST_RESULTS
    LAST_RESULTS = res
    outs = [res.results[c]["out"] for c in range(NCORES)]
    full = np.concatenate(outs, axis=0).reshape(B, C1, 64, 64)
    return full


if __name__ == "__main__":
    rng = np.random.default_rng(0)
    inputs = {
        "x1": rng.standard_normal((B, C1, 64, 64), dtype=np.float32),
        "x2": rng.standard_normal((B, C2, 64, 64), dtype=np.float32),
        "w_img": rng.standard_normal((C, C1), dtype=np.float32) / np.sqrt(C1),
        "w_txt": rng.standard_normal((C, C2), dtype=np.float32) / np.sqrt(C2),
        "w_out": rng.standard_normal((C1, C), dtype=np.float32) / np.sqrt(C),
        "gamma": rng.standard_normal(1).astype(np.float32),
    }
    out = kernel(**inputs)
    print(out.shape, out.dtype)
